# revision 7
# baseline (speedup 1.0000x reference)
"""Causal multi-head self-attention with RoPE on 8 Trainium2 NeuronCores.

Problem: B=2, S=2048, D=1024, 16 heads, d_k=64, fp32.

Sharding: core c -> (batch b = c//4, head-group g = c%4 of 4 heads).
Each core computes QKV projections for its batch (fp32r matmuls), RoPE,
causal attention for its 4 heads, and a partial output projection
y_partial = attn_out_g @ Wo[:, g_cols].T.  The host sums the 4 partials
per batch (the tensor-parallel all-reduce).

Device dataflow (per core):
  - activations kept head-dim-major: qT/kT [128, 2048] per head-pair
    (partitions = 2 heads x 64 dims, free = tokens).
  - RoPE: the interleaved even/odd rotation is re-expressed as rotate-half
    by permuting Wq/Wk rows per head on the host (scores are invariant to a
    shared permutation of q/k head dims). The 1/sqrt(d_k) scale is folded
    into Wq.  The cross-half combination uses a PE permutation matmul
    (swap 32-row halves) so every vector op stays partition-aligned:
      q' = q_tilde * CS + swap(q_tilde) * SN_signed
  - scores computed transposed: S^T[tk, tq] = kT_blk.T @ qT (so exp(S^T)
    tiles feed the PV matmul as the stationary operand with no transposes).
  - softmax denominator: V gets an appended ones-column, so the PV matmul
    accumulates both attn_out (rows 0..63) and the row-sum (row 64).
    exp() runs on the scalar engine straight out of PSUM; causal masking
    multiplies the four diagonal tiles by a host-built 0/1 mask.
  - normalization: reciprocal of the rowsum broadcast across partitions via
    a K=1 matmul with a ones vector, then one elementwise multiply.  Head
    B's normalized tile is moved to partitions 64..127 with an SBUF->SBUF
    DMA (engines cannot cross partitions).
"""
import os
import sys
import numpy as np

for _p in ("/opt/trn_rl_repo", "/root/.axon_site/_ro/trn_rl_repo"):
    if _p not in sys.path and os.path.isdir(_p):
        sys.path.insert(0, _p)

D = 1024
S = 2048
NH = 16
DK = 64
HG = 256          # head-group width per core (4 heads)
TB = 512          # token block
NT = S // TB      # 4
THETA = 10000.0

_CACHE = {}


def _build_nc():
    import concourse.tile as tile
    from concourse import bacc, mybir
    from concourse.masks import make_identity

    F32 = mybir.dt.float32
    F32R = mybir.dt.float32r
    EXP = mybir.ActivationFunctionType.Exp

    nc = bacc.Bacc("TRN2", target_bir_lowering=False)
    xt = nc.dram_tensor("xt", [D, S], F32R, kind="ExternalInput")
    wqt = nc.dram_tensor("wqt", [D, HG], F32R, kind="ExternalInput")
    wkt = nc.dram_tensor("wkt", [D, HG], F32R, kind="ExternalInput")
    wvt = nc.dram_tensor("wvt", [D, HG], F32R, kind="ExternalInput")
    wot = nc.dram_tensor("wot", [HG, D], F32R, kind="ExternalInput")
    swp = nc.dram_tensor("swp", [128, 128], F32R, kind="ExternalInput")
    cs = nc.dram_tensor("cs", [128, S], F32, kind="ExternalInput")
    sn = nc.dram_tensor("sn", [128, S], F32, kind="ExternalInput")
    msk = nc.dram_tensor("msk", [128, 4, TB], F32, kind="ExternalInput")
    yt = nc.dram_tensor("yt", [D, S], F32, kind="ExternalOutput")

    with tile.TileContext(nc) as tc:
        with tc.tile_pool(name="consts", bufs=1) as consts, \
             tc.tile_pool(name="persist", bufs=1) as persist:
            ident = consts.tile([128, 128], F32)
            make_identity(nc, ident)
            ones32 = consts.tile([128, 128], F32)
            nc.vector.memset(ones32, 1.0)
            ones_sb = consts.tile([128, 128], F32R)
            nc.vector.tensor_copy(ones_sb, ones32)
            swp_sb = consts.tile([128, 128], F32R)
            nc.sync.dma_start(out=swp_sb, in_=swp[:])
            cs_sb = consts.tile([128, S], F32)
            nc.sync.dma_start(out=cs_sb, in_=cs[:])
            sn_sb = consts.tile([128, S], F32)
            nc.sync.dma_start(out=sn_sb, in_=sn[:])
            msk_sb = consts.tile([128, 4, TB], F32, name="msk_sb")
            nc.sync.dma_start(out=msk_sb, in_=msk[:])

            qT = [persist.tile([128, S], F32R, name=f"qT{i}") for i in range(2)]
            kT = [persist.tile([128, S], F32R, name=f"kT{i}") for i in range(2)]
            # token-major V-hat per pair: per 128-token block, cols 0:64 head A,
            # 64 ones, 65:129 head B, 129 ones
            vh = [persist.tile([128, S // 128, 130], F32R, name=f"vh{i}")
                  for i in range(2)]
            attnT = [persist.tile([128, S], F32R, name=f"attnT{i}")
                     for i in range(2)]
            for ic in range(2):
                nc.vector.tensor_copy(vh[ic][:, :, 64], ones32[:, 0:S // 128])
                nc.vector.tensor_copy(vh[ic][:, :, 129], ones32[:, 0:S // 128])

            # ---------------- phase A: projections + RoPE + V transpose ------
            with tc.tile_pool(name="wghts", bufs=1) as wpool, \
                 tc.tile_pool(name="xts", bufs=2) as xpool, \
                 tc.tile_pool(name="ropet", bufs=4) as rpool, \
                 tc.tile_pool(name="ppa", bufs=3, space="PSUM") as ppa, \
                 tc.tile_pool(name="psw", bufs=2, space="PSUM") as psw, \
                 tc.tile_pool(name="ptr", bufs=2, space="PSUM") as ptr:
                wsbs = {}
                for name, src in (("q", wqt), ("k", wkt), ("v", wvt)):
                    wsb = wpool.tile([128, 8, HG], F32R, name=f"w{name}_sb")
                    for i in range(8):
                        nc.sync.dma_start(out=wsb[:, i, :],
                                          in_=src[128 * i:128 * (i + 1), :])
                    wsbs[name] = wsb

                for t in range(NT):
                    tsl = slice(t * TB, (t + 1) * TB)
                    x_sb = xpool.tile([128, 8, TB], F32R, tag="x")
                    for i in range(8):
                        nc.sync.dma_start(out=x_sb[:, i, :],
                                          in_=xt[128 * i:128 * (i + 1), tsl])
                    for ic in range(2):
                        csl = slice(ic * 128, (ic + 1) * 128)
                        for name, dest in (("q", qT), ("k", kT)):
                            ps = ppa.tile([128, TB], F32, tag="projps")
                            for i in range(8):
                                nc.tensor.matmul(ps, wsbs[name][:, i, csl],
                                                 x_sb[:, i, :],
                                                 start=(i == 0), stop=(i == 7))
                            # rope: dest = qt*CS + swap(qt)*SN_signed
                            qt_sb = rpool.tile([128, TB], F32R, tag="qt")
                            nc.vector.tensor_copy(qt_sb, ps)
                            sw_ps = psw.tile([128, TB], F32, tag="sw")
                            nc.tensor.matmul(sw_ps, swp_sb, qt_sb,
                                             start=True, stop=True)
                            t1 = rpool.tile([128, TB], F32, tag="t1")
                            t2 = rpool.tile([128, TB], F32, tag="t2")
                            nc.vector.tensor_mul(t1, qt_sb, cs_sb[:, tsl])
                            nc.vector.tensor_mul(t2, sw_ps, sn_sb[:, tsl])
                            nc.vector.tensor_add(dest[ic][:, tsl], t1, t2)
                        ps = ppa.tile([128, TB], F32, tag="projps")
                        for i in range(8):
                            nc.tensor.matmul(ps, wsbs["v"][:, i, csl],
                                             x_sb[:, i, :],
                                             start=(i == 0), stop=(i == 7))
                        vtmp = rpool.tile([128, TB], F32, tag="vtmp")
                        nc.vector.tensor_copy(vtmp, ps)
                        for s4 in range(4):
                            pt = ptr.tile([128, 128], F32, tag="ptr")
                            nc.tensor.transpose(pt,
                                                vtmp[:, 128 * s4:128 * (s4 + 1)],
                                                ident)
                            blk = t * 4 + s4
                            nc.vector.tensor_copy(vh[ic][:, blk, 0:64],
                                                  pt[:, 0:64])
                            nc.vector.tensor_copy(vh[ic][:, blk, 65:129],
                                                  pt[:, 64:128])

            # ---------------- phase B: attention -----------------------------
            with tc.tile_pool(name="expw", bufs=6) as epool, \
                 tc.tile_pool(name="rs", bufs=4) as rsp, \
                 tc.tile_pool(name="bshift", bufs=3) as bsh, \
                 tc.tile_pool(name="pw", bufs=3, space="PSUM") as pw, \
                 tc.tile_pool(name="pv", bufs=2, space="PSUM") as pvp:
                for ic in range(2):
                    for J in range(NT):
                        nblk = 4 * J + 4
                        Jsl = slice(J * TB, (J + 1) * TB)
                        pvA = pvp.tile([65, TB], F32, tag="pv")
                        pvB = pvp.tile([65, TB], F32, tag="pv")
                        for i in range(nblk):
                            tkb = slice(128 * i, 128 * (i + 1))
                            wps = pw.tile([128, 2, TB], F32, tag="w")
                            nc.tensor.matmul(wps[:, 0, :], kT[ic][0:64, tkb],
                                             qT[ic][0:64, Jsl],
                                             start=True, stop=True)
                            nc.tensor.matmul(wps[:, 1, :], kT[ic][64:128, tkb],
                                             qT[ic][64:128, Jsl],
                                             start=True, stop=True)
                            ew = epool.tile([128, 2, TB], F32R, tag="e")
                            nc.scalar.activation(ew, wps, EXP)
                            ii = i - 4 * J
                            if ii >= 0:
                                nc.vector.tensor_mul(ew[:, 0, :], ew[:, 0, :],
                                                     msk_sb[:, ii, :])
                                nc.vector.tensor_mul(ew[:, 1, :], ew[:, 1, :],
                                                     msk_sb[:, ii, :])
                            nc.tensor.matmul(pvA, vh[ic][:, i, 0:65],
                                             ew[:, 0, :], start=(i == 0),
                                             stop=(i == nblk - 1))
                            nc.tensor.matmul(pvB, vh[ic][:, i, 65:130],
                                             ew[:, 1, :], start=(i == 0),
                                             stop=(i == nblk - 1))
                        pvAs = rsp.tile([65, TB], F32, tag="pvs")
                        pvBs = rsp.tile([65, TB], F32, tag="pvs")
                        nc.vector.tensor_copy(pvAs, pvA)
                        nc.vector.tensor_copy(pvBs, pvB)
                        rA = rsp.tile([65, TB], F32R, tag="r")
                        rB = rsp.tile([65, TB], F32R, tag="r")
                        with nc.allow_low_precision(
                                reason="fp32r rowsum reciprocal (12-bit "
                                       "mantissa is plenty for softmax norm)"):
                            nc.vector.reciprocal(rA[64:65, :], pvAs[64:65, :])
                            nc.vector.reciprocal(rB[64:65, :], pvBs[64:65, :])
                        bc = pw.tile([128, 2, TB], F32, tag="w")
                        nc.tensor.matmul(bc[0:64, 0, :], ones_sb[64:65, 0:64],
                                         rA[64:65, :], start=True, stop=True)
                        nc.tensor.matmul(bc[0:64, 1, :], ones_sb[64:65, 0:64],
                                         rB[64:65, :], start=True, stop=True)
                        nc.vector.tensor_mul(attnT[ic][0:64, Jsl], pvAs[0:64, :],
                                             bc[0:64, 0, :])
                        tmpB = bsh.tile([64, TB], F32R, tag="tmpB")
                        nc.vector.tensor_mul(tmpB, pvBs[0:64, :], bc[0:64, 1, :])
                        nc.sync.dma_start(out=attnT[ic][64:128, Jsl], in_=tmpB)

            # ---------------- phase C: output projection ---------------------
            with tc.tile_pool(name="wo", bufs=1) as wop, \
                 tc.tile_pool(name="yst", bufs=3) as ysp, \
                 tc.tile_pool(name="pyp", bufs=2, space="PSUM") as pyp:
                wo_sb = wop.tile([128, 2, D], F32R)
                nc.sync.dma_start(out=wo_sb[:, 0, :], in_=wot[0:128, :])
                nc.sync.dma_start(out=wo_sb[:, 1, :], in_=wot[128:256, :])
                for j in range(8):
                    jsl = slice(128 * j, 128 * (j + 1))
                    for t in range(NT):
                        tsl = slice(t * TB, (t + 1) * TB)
                        yp = pyp.tile([128, TB], F32, tag="y")
                        for ic in range(2):
                            nc.tensor.matmul(yp, wo_sb[:, ic, jsl],
                                             attnT[ic][:, tsl],
                                             start=(ic == 0), stop=(ic == 1))
                        ys = ysp.tile([128, TB], F32, tag="ys")
                        nc.vector.tensor_copy(ys, yp)
                        nc.sync.dma_start(out=yt[jsl, tsl], in_=ys)

    nc.compile()
    return nc


def _host_prep(in_features, token_positions, Wq, Wk, Wv, Wo):
    X = np.ascontiguousarray(np.asarray(in_features, dtype=np.float32))
    pos = np.asarray(token_positions)
    Wq = np.asarray(Wq, dtype=np.float32)
    Wk = np.asarray(Wk, dtype=np.float32)
    Wv = np.asarray(Wv, dtype=np.float32)
    Wo = np.asarray(Wo, dtype=np.float32)

    freq = 1.0 / np.power(np.float32(THETA),
                          np.arange(0, DK, 2, dtype=np.float32) / DK)
    freqs = np.outer(pos.astype(np.float32), freq)      # [S, 32]
    CS = np.tile(np.cos(freqs).T.astype(np.float32), (4, 1))  # [128, S]
    sinT = np.sin(freqs).T.astype(np.float32)           # [32, S]
    SN = np.concatenate([-sinT, sinT, -sinT, sinT], axis=0)   # signed [128, S]

    # swap matrix: exchanges 32-row halves within each 64-row head block
    swap = np.arange(128)
    swap = np.where(swap % 64 < 32, swap + 32, swap - 32)
    SWP = np.zeros((128, 128), np.float32)
    SWP[swap, np.arange(128)] = 1.0

    # causal mask for the 4 diagonal-region tk blocks of a tq block:
    # msk[p, ii, f] = 1 iff p <= f - 128*ii
    p_idx = np.arange(128)[:, None, None]
    ii_idx = np.arange(4)[None, :, None]
    f_idx = np.arange(TB)[None, None, :]
    MSK = (p_idx <= f_idx - 128 * ii_idx).astype(np.float32)

    perm = np.concatenate([np.arange(0, DK, 2), np.arange(1, DK, 2)])

    def prep_qk(W, scale):
        out = {}
        for g in range(4):
            rows = [W[h * DK:(h + 1) * DK][perm] * scale
                    for h in range(4 * g, 4 * g + 4)]
            out[g] = np.ascontiguousarray(np.concatenate(rows, axis=0).T)
        return out

    wqts = prep_qk(Wq, np.float32(1.0 / 8.0))
    wkts = prep_qk(Wk, np.float32(1.0))
    wvts = {g: np.ascontiguousarray(Wv[HG * g:HG * (g + 1)].T) for g in range(4)}
    wots = {g: np.ascontiguousarray(Wo[:, HG * g:HG * (g + 1)].T)
            for g in range(4)}
    xts = {b: np.ascontiguousarray(X[b].T) for b in range(2)}

    in_maps = []
    for c in range(8):
        b, g = c // 4, c % 4
        in_maps.append({
            "xt": xts[b], "wqt": wqts[g], "wkt": wkts[g], "wvt": wvts[g],
            "wot": wots[g], "swp": SWP, "cs": CS, "sn": SN, "msk": MSK,
        })
    return in_maps


def kernel(in_features, token_positions, Wq, Wk, Wv, Wo):
    from concourse.bass_utils import run_bass_kernel_spmd

    if "nc" not in _CACHE:
        _CACHE["nc"] = _build_nc()
    nc = _CACHE["nc"]

    in_maps = _host_prep(in_features, token_positions, Wq, Wk, Wv, Wo)
    res = run_bass_kernel_spmd(nc, in_maps, list(range(8)))

    B = np.asarray(in_features).shape[0]
    y = np.zeros((B, S, D), np.float32)
    for c in range(8):
        b = c // 4
        y[b] += res.results[c]["yt"].T
    return y


# revision 8
# speedup vs baseline: 11441.0520x; 11441.0520x over previous
"""Causal multi-head self-attention with RoPE on 8 Trainium2 NeuronCores.

Problem: B=2, S=2048, D=1024, 16 heads, d_k=64, fp32.

Sharding: core c -> (batch b = c//4, head-group g = c%4 of 4 heads).
Each core computes QKV projections for its batch (fp32r matmuls), RoPE,
causal attention for its 4 heads, and a partial output projection
y_partial = attn_out_g @ Wo[:, g_cols].T.  The host sums the 4 partials
per batch (the tensor-parallel all-reduce).

Device dataflow (per core):
  - activations kept head-dim-major: qT/kT [128, 2048] per head-pair
    (partitions = 2 heads x 64 dims, free = tokens).
  - RoPE: the interleaved even/odd rotation is re-expressed as rotate-half
    by permuting Wq/Wk rows per head on the host (scores are invariant to a
    shared permutation of q/k head dims). The 1/sqrt(d_k) scale is folded
    into Wq.  The cross-half combination uses a PE permutation matmul
    (swap 32-row halves) so every vector op stays partition-aligned:
      q' = q_tilde * CS + swap(q_tilde) * SN_signed
  - scores computed transposed: S^T[tk, tq] = kT_blk.T @ qT (so exp(S^T)
    tiles feed the PV matmul as the stationary operand with no transposes).
  - softmax denominator: V gets an appended ones-column, so the PV matmul
    accumulates both attn_out (rows 0..63) and the row-sum (row 64).
    exp() runs on the scalar engine straight out of PSUM; causal masking
    multiplies the four diagonal tiles by a host-built 0/1 mask.
  - normalization: reciprocal of the rowsum broadcast across partitions via
    a K=1 matmul with a ones vector, then one elementwise multiply.  Head
    B's normalized tile is moved to partitions 64..127 with an SBUF->SBUF
    DMA (engines cannot cross partitions).
"""
import os
import sys
import numpy as np

for _p in ("/opt/trn_rl_repo", "/root/.axon_site/_ro/trn_rl_repo"):
    if _p not in sys.path and os.path.isdir(_p):
        sys.path.insert(0, _p)

D = 1024
S = 2048
NH = 16
DK = 64
HG = 256          # head-group width per core (4 heads)
TB = 512          # token block
NT = S // TB      # 4
THETA = 10000.0

_CACHE = {}


def _build_nc(reps=1):
    import concourse.tile as tile
    from concourse import bacc, mybir
    from concourse.masks import make_identity

    F32 = mybir.dt.float32
    F32R = mybir.dt.float32r
    EXP = mybir.ActivationFunctionType.Exp

    nc = bacc.Bacc("TRN2", target_bir_lowering=False)
    xt = nc.dram_tensor("xt", [D, S], F32R, kind="ExternalInput")
    wqt = nc.dram_tensor("wqt", [D, HG], F32R, kind="ExternalInput")
    wkt = nc.dram_tensor("wkt", [D, HG], F32R, kind="ExternalInput")
    wvt = nc.dram_tensor("wvt", [D, HG], F32R, kind="ExternalInput")
    wot = nc.dram_tensor("wot", [HG, D], F32R, kind="ExternalInput")
    swp = nc.dram_tensor("swp", [128, 128], F32R, kind="ExternalInput")
    cs = nc.dram_tensor("cs", [128, S], F32, kind="ExternalInput")
    sn = nc.dram_tensor("sn", [128, S], F32, kind="ExternalInput")
    msk = nc.dram_tensor("msk", [128, 4, TB], F32, kind="ExternalInput")
    yt = nc.dram_tensor("yt", [D, S], F32, kind="ExternalOutput")

    from contextlib import nullcontext

    with tile.TileContext(nc) as tc:
        with tc.tile_pool(name="consts", bufs=1) as consts, \
             tc.tile_pool(name="persist", bufs=1) as persist:
            ident = consts.tile([128, 128], F32)
            make_identity(nc, ident)
            ones32 = consts.tile([128, 128], F32)
            nc.vector.memset(ones32, 1.0)
            ones_sb = consts.tile([128, 128], F32R)
            nc.vector.tensor_copy(ones_sb, ones32)
            swp_sb = consts.tile([128, 128], F32R)
            nc.sync.dma_start(out=swp_sb, in_=swp[:])
            cs_sb = consts.tile([128, S], F32)
            nc.sync.dma_start(out=cs_sb, in_=cs[:])
            sn_sb = consts.tile([128, S], F32)
            nc.sync.dma_start(out=sn_sb, in_=sn[:])
            msk_sb = consts.tile([128, 4, TB], F32, name="msk_sb")
            nc.sync.dma_start(out=msk_sb, in_=msk[:])

            qT = [persist.tile([128, S], F32R, name=f"qT{i}") for i in range(2)]
            kT = [persist.tile([128, S], F32R, name=f"kT{i}") for i in range(2)]
            # token-major V-hat per pair: per 128-token block, cols 0:64 head A,
            # 64 ones, 65:129 head B, 129 ones
            vh = [persist.tile([128, S // 128, 130], F32R, name=f"vh{i}")
                  for i in range(2)]
            attnT = [persist.tile([128, S], F32R, name=f"attnT{i}")
                     for i in range(2)]
            for ic in range(2):
                nc.vector.tensor_copy(vh[ic][:, :, 64], ones32[:, 0:S // 128])
                nc.vector.tensor_copy(vh[ic][:, :, 129], ones32[:, 0:S // 128])

            # optional on-device repeat loop for benchmarking (reps>1)
            loop_cm = tc.For_i(0, reps, 1) if reps != 1 else nullcontext()
            with loop_cm:
                _phases(nc, tc, tile, mybir, locals())
    nc.compile()
    return nc


def _phases(nc, tc, tile, mybir, env):
    F32 = mybir.dt.float32
    F32R = mybir.dt.float32r
    EXP = mybir.ActivationFunctionType.Exp
    xt, wqt, wkt, wvt, wot = env["xt"], env["wqt"], env["wkt"], env["wvt"], env["wot"]
    yt = env["yt"]
    ident, ones_sb, swp_sb = env["ident"], env["ones_sb"], env["swp_sb"]
    cs_sb, sn_sb, msk_sb = env["cs_sb"], env["sn_sb"], env["msk_sb"]
    qT, kT, vh, attnT = env["qT"], env["kT"], env["vh"], env["attnT"]
    if True:
        if True:
            # ---------------- phase A: projections + RoPE + V transpose ------
            with tc.tile_pool(name="wghts", bufs=1) as wpool, \
                 tc.tile_pool(name="xts", bufs=2) as xpool, \
                 tc.tile_pool(name="ropet", bufs=4) as rpool, \
                 tc.tile_pool(name="ppa", bufs=3, space="PSUM") as ppa, \
                 tc.tile_pool(name="psw", bufs=2, space="PSUM") as psw, \
                 tc.tile_pool(name="ptr", bufs=2, space="PSUM") as ptr:
                wsbs = {}
                for name, src in (("q", wqt), ("k", wkt), ("v", wvt)):
                    wsb = wpool.tile([128, 8, HG], F32R, name=f"w{name}_sb")
                    for i in range(8):
                        nc.sync.dma_start(out=wsb[:, i, :],
                                          in_=src[128 * i:128 * (i + 1), :])
                    wsbs[name] = wsb

                for t in range(NT):
                    tsl = slice(t * TB, (t + 1) * TB)
                    x_sb = xpool.tile([128, 8, TB], F32R, tag="x")
                    for i in range(8):
                        nc.sync.dma_start(out=x_sb[:, i, :],
                                          in_=xt[128 * i:128 * (i + 1), tsl])
                    for ic in range(2):
                        csl = slice(ic * 128, (ic + 1) * 128)
                        for name, dest in (("q", qT), ("k", kT)):
                            ps = ppa.tile([128, TB], F32, tag="projps")
                            for i in range(8):
                                nc.tensor.matmul(ps, wsbs[name][:, i, csl],
                                                 x_sb[:, i, :],
                                                 start=(i == 0), stop=(i == 7))
                            # rope: dest = qt*CS + swap(qt)*SN_signed
                            qt_sb = rpool.tile([128, TB], F32R, tag="qt")
                            nc.vector.tensor_copy(qt_sb, ps)
                            sw_ps = psw.tile([128, TB], F32, tag="sw")
                            nc.tensor.matmul(sw_ps, swp_sb, qt_sb,
                                             start=True, stop=True)
                            t1 = rpool.tile([128, TB], F32, tag="t1")
                            t2 = rpool.tile([128, TB], F32, tag="t2")
                            nc.vector.tensor_mul(t1, qt_sb, cs_sb[:, tsl])
                            nc.vector.tensor_mul(t2, sw_ps, sn_sb[:, tsl])
                            nc.vector.tensor_add(dest[ic][:, tsl], t1, t2)
                        ps = ppa.tile([128, TB], F32, tag="projps")
                        for i in range(8):
                            nc.tensor.matmul(ps, wsbs["v"][:, i, csl],
                                             x_sb[:, i, :],
                                             start=(i == 0), stop=(i == 7))
                        vtmp = rpool.tile([128, TB], F32, tag="vtmp")
                        nc.vector.tensor_copy(vtmp, ps)
                        for s4 in range(4):
                            pt = ptr.tile([128, 128], F32, tag="ptr")
                            nc.tensor.transpose(pt,
                                                vtmp[:, 128 * s4:128 * (s4 + 1)],
                                                ident)
                            blk = t * 4 + s4
                            nc.vector.tensor_copy(vh[ic][:, blk, 0:64],
                                                  pt[:, 0:64])
                            nc.vector.tensor_copy(vh[ic][:, blk, 65:129],
                                                  pt[:, 64:128])

            # ---------------- phase B: attention -----------------------------
            with tc.tile_pool(name="expw", bufs=6) as epool, \
                 tc.tile_pool(name="rs", bufs=4) as rsp, \
                 tc.tile_pool(name="bshift", bufs=3) as bsh, \
                 tc.tile_pool(name="pw", bufs=3, space="PSUM") as pw, \
                 tc.tile_pool(name="pv", bufs=2, space="PSUM") as pvp:
                for ic in range(2):
                    for J in range(NT):
                        nblk = 4 * J + 4
                        Jsl = slice(J * TB, (J + 1) * TB)
                        pvA = pvp.tile([65, TB], F32, tag="pv")
                        pvB = pvp.tile([65, TB], F32, tag="pv")
                        for i in range(nblk):
                            tkb = slice(128 * i, 128 * (i + 1))
                            wps = pw.tile([128, 2, TB], F32, tag="w")
                            nc.tensor.matmul(wps[:, 0, :], kT[ic][0:64, tkb],
                                             qT[ic][0:64, Jsl],
                                             start=True, stop=True)
                            nc.tensor.matmul(wps[:, 1, :], kT[ic][64:128, tkb],
                                             qT[ic][64:128, Jsl],
                                             start=True, stop=True)
                            ew = epool.tile([128, 2, TB], F32R, tag="e")
                            nc.scalar.activation(ew, wps, EXP)
                            ii = i - 4 * J
                            if ii >= 0:
                                nc.vector.tensor_mul(ew[:, 0, :], ew[:, 0, :],
                                                     msk_sb[:, ii, :])
                                nc.vector.tensor_mul(ew[:, 1, :], ew[:, 1, :],
                                                     msk_sb[:, ii, :])
                            nc.tensor.matmul(pvA, vh[ic][:, i, 0:65],
                                             ew[:, 0, :], start=(i == 0),
                                             stop=(i == nblk - 1))
                            nc.tensor.matmul(pvB, vh[ic][:, i, 65:130],
                                             ew[:, 1, :], start=(i == 0),
                                             stop=(i == nblk - 1))
                        pvAs = rsp.tile([65, TB], F32, tag="pvs")
                        pvBs = rsp.tile([65, TB], F32, tag="pvs")
                        nc.vector.tensor_copy(pvAs, pvA)
                        nc.vector.tensor_copy(pvBs, pvB)
                        rA = rsp.tile([65, TB], F32R, tag="r")
                        rB = rsp.tile([65, TB], F32R, tag="r")
                        with nc.allow_low_precision(
                                reason="fp32r rowsum reciprocal (12-bit "
                                       "mantissa is plenty for softmax norm)"):
                            nc.vector.reciprocal(rA[64:65, :], pvAs[64:65, :])
                            nc.vector.reciprocal(rB[64:65, :], pvBs[64:65, :])
                        bc = pw.tile([128, 2, TB], F32, tag="w")
                        nc.tensor.matmul(bc[0:64, 0, :], ones_sb[64:65, 0:64],
                                         rA[64:65, :], start=True, stop=True)
                        nc.tensor.matmul(bc[0:64, 1, :], ones_sb[64:65, 0:64],
                                         rB[64:65, :], start=True, stop=True)
                        nc.vector.tensor_mul(attnT[ic][0:64, Jsl], pvAs[0:64, :],
                                             bc[0:64, 0, :])
                        tmpB = bsh.tile([64, TB], F32R, tag="tmpB")
                        nc.vector.tensor_mul(tmpB, pvBs[0:64, :], bc[0:64, 1, :])
                        nc.sync.dma_start(out=attnT[ic][64:128, Jsl], in_=tmpB)

            # ---------------- phase C: output projection ---------------------
            with tc.tile_pool(name="wo", bufs=1) as wop, \
                 tc.tile_pool(name="yst", bufs=3) as ysp, \
                 tc.tile_pool(name="pyp", bufs=2, space="PSUM") as pyp:
                wo_sb = wop.tile([128, 2, D], F32R)
                nc.sync.dma_start(out=wo_sb[:, 0, :], in_=wot[0:128, :])
                nc.sync.dma_start(out=wo_sb[:, 1, :], in_=wot[128:256, :])
                for j in range(8):
                    jsl = slice(128 * j, 128 * (j + 1))
                    for t in range(NT):
                        tsl = slice(t * TB, (t + 1) * TB)
                        yp = pyp.tile([128, TB], F32, tag="y")
                        for ic in range(2):
                            nc.tensor.matmul(yp, wo_sb[:, ic, jsl],
                                             attnT[ic][:, tsl],
                                             start=(ic == 0), stop=(ic == 1))
                        ys = ysp.tile([128, TB], F32, tag="ys")
                        nc.vector.tensor_copy(ys, yp)
                        nc.sync.dma_start(out=yt[jsl, tsl], in_=ys)


def _host_prep(in_features, token_positions, Wq, Wk, Wv, Wo):
    X = np.ascontiguousarray(np.asarray(in_features, dtype=np.float32))
    pos = np.asarray(token_positions)
    Wq = np.asarray(Wq, dtype=np.float32)
    Wk = np.asarray(Wk, dtype=np.float32)
    Wv = np.asarray(Wv, dtype=np.float32)
    Wo = np.asarray(Wo, dtype=np.float32)

    freq = 1.0 / np.power(np.float32(THETA),
                          np.arange(0, DK, 2, dtype=np.float32) / DK)
    freqs = np.outer(pos.astype(np.float32), freq)      # [S, 32]
    CS = np.tile(np.cos(freqs).T.astype(np.float32), (4, 1))  # [128, S]
    sinT = np.sin(freqs).T.astype(np.float32)           # [32, S]
    SN = np.concatenate([-sinT, sinT, -sinT, sinT], axis=0)   # signed [128, S]

    # swap matrix: exchanges 32-row halves within each 64-row head block
    swap = np.arange(128)
    swap = np.where(swap % 64 < 32, swap + 32, swap - 32)
    SWP = np.zeros((128, 128), np.float32)
    SWP[swap, np.arange(128)] = 1.0

    # causal mask for the 4 diagonal-region tk blocks of a tq block:
    # msk[p, ii, f] = 1 iff p <= f - 128*ii
    p_idx = np.arange(128)[:, None, None]
    ii_idx = np.arange(4)[None, :, None]
    f_idx = np.arange(TB)[None, None, :]
    MSK = (p_idx <= f_idx - 128 * ii_idx).astype(np.float32)

    perm = np.concatenate([np.arange(0, DK, 2), np.arange(1, DK, 2)])

    def prep_qk(W, scale):
        out = {}
        for g in range(4):
            rows = [W[h * DK:(h + 1) * DK][perm] * scale
                    for h in range(4 * g, 4 * g + 4)]
            out[g] = np.ascontiguousarray(np.concatenate(rows, axis=0).T)
        return out

    wqts = prep_qk(Wq, np.float32(1.0 / 8.0))
    wkts = prep_qk(Wk, np.float32(1.0))
    wvts = {g: np.ascontiguousarray(Wv[HG * g:HG * (g + 1)].T) for g in range(4)}
    wots = {g: np.ascontiguousarray(Wo[:, HG * g:HG * (g + 1)].T)
            for g in range(4)}
    xts = {b: np.ascontiguousarray(X[b].T) for b in range(2)}

    in_maps = []
    for c in range(8):
        b, g = c // 4, c % 4
        in_maps.append({
            "xt": xts[b], "wqt": wqts[g], "wkt": wkts[g], "wvt": wvts[g],
            "wot": wots[g], "swp": SWP, "cs": CS, "sn": SN, "msk": MSK,
        })
    return in_maps


def kernel(in_features, token_positions, Wq, Wk, Wv, Wo):
    from concourse.bass_utils import run_bass_kernel_spmd

    if "nc" not in _CACHE:
        _CACHE["nc"] = _build_nc()
    nc = _CACHE["nc"]

    in_maps = _host_prep(in_features, token_positions, Wq, Wk, Wv, Wo)
    res = run_bass_kernel_spmd(nc, in_maps, list(range(8)))

    B = np.asarray(in_features).shape[0]
    y = np.zeros((B, S, D), np.float32)
    for c in range(8):
        b = c // 4
        y[b] += res.results[c]["yt"].T
    return y
